# revision 15
# baseline (speedup 1.0000x reference)
"""Trainium2 Bass kernel for AnatomicalBiasedAttention.

Reference computation (fp32, B=2, NQ=NK=2048, D=1024, H=16, DH=64):
    Q = query @ Wq.T + bq ; K = key @ Wk.T + bk ; V = value @ Wv.T + bv
    scores = QK^T/sqrt(DH) + vessel_bias  (bias broadcast over heads)
    attn = softmax(scores, -1)
    out = (attn @ V) @ Wo.T + bo
    returns (out, attn)

Sharding: tensor-parallel over heads, 2 heads per core on 8 NeuronCores.
Each core projects its 2 heads' Q/K/V (pre-transposed bf16 operands are
prepared on the host), computes biased softmax attention for its heads,
writes its attn shard, and produces a partial output projection; the host
sums the 8 partials (TP unshard) and concatenates attn shards.
"""

import math
import numpy as np
import ml_dtypes
from contextlib import ExitStack

import concourse.bass as bass
import concourse.tile as tile
from concourse import bacc, mybir
from concourse.bass_utils import run_bass_kernel_spmd

BF16 = ml_dtypes.bfloat16
FP32 = mybir.dt.float32
BF = mybir.dt.bfloat16
AF = mybir.ActivationFunctionType
ts = bass.ts

N_CORES = 8
B = 2
NQ = 2048
NK = 2048
D = 1024
H = 16
DH = 64
HPC = H // N_CORES  # heads per core = 2


def build_nc(b=B, nq=NQ, nk=NK, d=D, hpc=HPC, dh=DH):
    """Build the per-core Bass graph (SPMD: all 8 cores run this graph).

    DRAM parameters (per-core shards, prepared by the host):
      qT, kT, vT   [d, b*ntok]  bf16   full transposed activations (replicated)
      wqT, wkT, wvT [d, hpc*dh] bf16   per-core head-slice of W.T (wqT pre-scaled 1/sqrt(dh))
      bq, bk       [hpc*dh, 1]  f32    per-core bias slices (bq pre-scaled)
      bv           [1, hpc*dh]  bf16
      biasn        [b, nq, nk]  bf16   vessel bias, natural layout (replicated)
      woT          [hpc*dh, d]  bf16   per-core rows of Wo.T
      bo8          [1, d]       bf16   bo / n_cores
      ident        [128, 128]   bf16   identity matrix
    Outputs:
      attn_sh [b, hpc, nq, nk] bf16
      partial [b, nq, d]       bf16
    """
    assert hpc * dh == 128
    NTOK_Q = b * nq
    NTOK_K = b * nk
    DT = d // 128       # D tiles
    TCH = 512           # token chunk for projections
    KC = min(512, nk)   # k chunk for scores
    OC = min(512, d)    # out-proj column chunk

    nc = bacc.Bacc("TRN2", target_bir_lowering=False, debug=False, num_devices=N_CORES)

    qT = nc.dram_tensor("qT", [d, NTOK_Q], BF, kind="ExternalInput").ap()
    kT = nc.dram_tensor("kT", [d, NTOK_K], BF, kind="ExternalInput").ap()
    vT = nc.dram_tensor("vT", [d, NTOK_K], BF, kind="ExternalInput").ap()
    wqT = nc.dram_tensor("wqT", [d, 128], BF, kind="ExternalInput").ap()
    wkT = nc.dram_tensor("wkT", [d, 128], BF, kind="ExternalInput").ap()
    wvT = nc.dram_tensor("wvT", [d, 128], BF, kind="ExternalInput").ap()
    bq = nc.dram_tensor("bq", [128, 1], FP32, kind="ExternalInput").ap()
    bk = nc.dram_tensor("bk", [128, 1], FP32, kind="ExternalInput").ap()
    bv = nc.dram_tensor("bv", [1, 128], BF, kind="ExternalInput").ap()
    biasn = nc.dram_tensor("biasn", [b, nq, nk], BF, kind="ExternalInput").ap()
    woT = nc.dram_tensor("woT", [128, d], BF, kind="ExternalInput").ap()
    ident = nc.dram_tensor("ident", [128, 128], BF, kind="ExternalInput").ap()

    attn_sh = nc.dram_tensor("attn_sh", [b, hpc, nq, nk], BF, kind="ExternalOutput").ap()
    partial = nc.dram_tensor("partial", [b, nq, d], BF, kind="ExternalOutput").ap()

    with tile.TileContext(nc) as tc, ExitStack() as ctx:
        # ---- constants resident in SBUF ----
        cpool = ctx.enter_context(tc.tile_pool(name="consts", bufs=1))
        wq_sb = cpool.tile([128, DT * 128], BF, tag="wq")
        nc.sync.dma_start(wq_sb[:].rearrange("p (a f) -> p a f", f=128),
                          wqT.rearrange("(a p) f -> p a f", p=128))
        wk_sb = cpool.tile([128, DT * 128], BF, tag="wk")
        nc.sync.dma_start(wk_sb[:].rearrange("p (a f) -> p a f", f=128),
                          wkT.rearrange("(a p) f -> p a f", p=128))
        wv_sb = cpool.tile([128, DT * 128], BF, tag="wv")
        nc.sync.dma_start(wv_sb[:].rearrange("p (a f) -> p a f", f=128),
                          wvT.rearrange("(a p) f -> p a f", p=128))
        wo_sb = cpool.tile([128, d], BF, tag="wo")
        nc.sync.dma_start(wo_sb[:], woT[:])
        bq_sb = cpool.tile([128, 1], FP32, tag="bq")
        nc.sync.dma_start(bq_sb[:], bq[:])
        bk_sb = cpool.tile([128, 1], FP32, tag="bk")
        nc.sync.dma_start(bk_sb[:], bk[:])
        bv_sb = cpool.tile([1, 128], BF, tag="bv")
        nc.sync.dma_start(bv_sb[:], bv[:])
        id_sb = cpool.tile([128, 128], BF, tag="ident")
        nc.sync.dma_start(id_sb[:], ident[:])
        ones_sb = cpool.tile([1, 128], BF, tag="ones")
        nc.vector.memset(ones_sb[:], 1.0)

        # persistent activations (bf16)
        apool = ctx.enter_context(tc.tile_pool(name="acts", bufs=1))
        QT_sb = apool.tile([128, NTOK_Q], BF, tag="QT")    # [2 heads x dh, tokens]
        KT_sb = apool.tile([128, NTOK_K], BF, tag="KT")
        V_sb = apool.tile([128, NTOK_K], BF, tag="V")      # natural: block t = tokens 128t..
        OT_sb = apool.tile([128, NTOK_Q], BF, tag="OT")    # attn-out^T, rows=2*dh concat

        # ---- stage A: projections ----
        with ExitStack() as sa:
            inp = sa.enter_context(tc.tile_pool(name="proj_in", bufs=2))
            psa = sa.enter_context(tc.tile_pool(name="proj_ps", bufs=2, space="PSUM"))
            psv = sa.enter_context(tc.tile_pool(name="proj_psv", bufs=2, space="PSUM"))
            for tch in range(max(NTOK_Q, NTOK_K) // TCH):
                if tch < NTOK_Q // TCH:
                    qin = inp.tile([128, DT * TCH], BF, tag="qin")
                    nc.sync.dma_start(
                        qin[:].rearrange("p (a f) -> p a f", a=DT),
                        qT[:, ts(tch, TCH)].rearrange("(a p) f -> p a f", p=128))
                    ps_q = psa.tile([128, TCH], FP32, tag="psq")
                    for dt_ in range(DT):
                        nc.tensor.matmul(ps_q[:], wq_sb[:, ts(dt_, 128)],
                                         qin[:, ts(dt_, TCH)],
                                         start=(dt_ == 0), stop=(dt_ == DT - 1))
                    nc.vector.tensor_scalar_add(QT_sb[:, ts(tch, TCH)], ps_q[:], bq_sb[:])
                if tch >= NTOK_K // TCH:
                    continue
                kin = inp.tile([128, DT * TCH], BF, tag="kin")
                nc.sync.dma_start(
                    kin[:].rearrange("p (a f) -> p a f", a=DT),
                    kT[:, ts(tch, TCH)].rearrange("(a p) f -> p a f", p=128))
                vin = inp.tile([128, DT * TCH], BF, tag="vin")
                nc.sync.dma_start(
                    vin[:].rearrange("p (a f) -> p a f", a=DT),
                    vT[:, ts(tch, TCH)].rearrange("(a p) f -> p a f", p=128))

                ps_k = psa.tile([128, TCH], FP32, tag="psq")
                for dt_ in range(DT):
                    nc.tensor.matmul(ps_k[:], wk_sb[:, ts(dt_, 128)],
                                     kin[:, ts(dt_, TCH)],
                                     start=(dt_ == 0), stop=(dt_ == DT - 1))
                nc.vector.tensor_scalar_add(KT_sb[:, ts(tch, TCH)], ps_k[:], bk_sb[:])

                ps_v = psv.tile([128, TCH], FP32, tag="psv")
                for tb in range(TCH // 128):
                    for dt_ in range(DT):
                        nc.tensor.matmul(ps_v[:, ts(tb, 128)],
                                         vin[:, bass.ds(dt_ * TCH + tb * 128, 128)],
                                         wv_sb[:, ts(dt_, 128)],
                                         start=(dt_ == 0), stop=False)
                    nc.tensor.matmul(ps_v[:, ts(tb, 128)], ones_sb[:], bv_sb[:],
                                     start=False, stop=True)
                nc.vector.tensor_copy(V_sb[:, ts(tch, TCH)], ps_v[:])

        # ---- stage B: attention ----
        # scores = QK^T/sqrt(dh) in PSUM; softmax via exp(s)*exp(bias)
        # (host supplies eb = exp(vessel_bias)); sums via fused accum_out.
        # attn@V consumes PE-transposed normalized attn in 512-wide groups.
        with ExitStack() as sb:
            bpool = sb.enter_context(tc.tile_pool(name="biasp", bufs=6))
            atpool = sb.enter_context(tc.tile_pool(name="atpool", bufs=5))
            smpool = sb.enter_context(tc.tile_pool(name="smalls", bufs=8))
            aTpool = sb.enter_context(tc.tile_pool(name="aT", bufs=2))
            ppool = sb.enter_context(tc.tile_pool(name="pp", bufs=4))
            pss = sb.enter_context(tc.tile_pool(name="ps_s", bufs=2, space="PSUM"))
            pstr = sb.enter_context(tc.tile_pool(name="ps_tr", bufs=2, space="PSUM"))
            pso = sb.enter_context(tc.tile_pool(name="ps_o", bufs=2, space="PSUM"))
            QG = min(4, nq // 128)  # q-tiles per attn@V group
            for bb in range(b):
                for qg in range(nq // (128 * QG)):
                    eb_tiles = []
                    for qi in range(QG):
                        qt = qg * QG + qi
                        ebt = bpool.tile([128, nk], BF, tag="bias")
                        nc.sync.dma_start(ebt[:], biasn[bb, ts(qt, 128), :])
                        eb_tiles.append(ebt)
                    ps_o2 = pso.tile([128, QG * 128], FP32, tag="pso")
                    at_tiles = [[] for _ in range(hpc)]
                    SH = min(1024, nk)  # psum half-row width
                    for qi in range(QG):
                        qt = qg * QG + qi
                        # row-packed scores: both heads issued adjacently so the
                        # K=64 matmuls can run concurrently on row-groups 0-1 / 2-3;
                        # vessel bias accumulated into PSUM via identity matmul;
                        # exp evacuates PSUM on ACT with fused row-sum accumulation.
                        U_tiles = [atpool.tile([128, nk], BF, tag=f"at{_h}", name=f"U{_h}") for _h in range(hpc)]
                        ss_h = [smpool.tile([128, 2], FP32, tag="ssum", name=f"ss{_h}") for _h in range(hpc)]
                        for sh in range(nk // SH):
                            ps_h = [pss.tile([128, SH], FP32, tag="pss", name=f"psh{_h}") for _h in range(hpc)]
                            for kc in range(SH // KC):
                                for h in range(hpc):
                                    nc.tensor.matmul(
                                        ps_h[h][:, ts(kc, KC)],
                                        QT_sb[bass.ds(h * dh, dh), bass.ds(bb * nq + qt * 128, 128)],
                                        KT_sb[bass.ds(h * dh, dh), bass.ds(bb * nk + sh * SH + kc * KC, KC)],
                                        start=True, stop=False)
                                for h in range(hpc):
                                    nc.tensor.matmul(
                                        ps_h[h][:, ts(kc, KC)], id_sb[:],
                                        eb_tiles[qi][:, bass.ds(sh * SH + kc * KC, KC)],
                                        start=False, stop=True)
                            for h in range(hpc):
                                nc.scalar.activation(U_tiles[h][:, ts(sh, SH)], ps_h[h][:],
                                                     AF.Exp, accum_out=ss_h[h][:, sh:sh + 1])
                        for h in range(hpc):
                            U_sb = U_tiles[h]
                            rsum = smpool.tile([128, 1], FP32, tag="rsum")
                            if nk // SH > 1:
                                ssum = smpool.tile([128, 1], FP32, tag="ssa")
                                nc.vector.tensor_add(ssum[:], ss_h[h][:, 0:1], ss_h[h][:, 1:2])
                                nc.vector.reciprocal(rsum[:], ssum[:])
                            else:
                                nc.vector.reciprocal(rsum[:], ss_h[h][:, 0:1])
                            nc.vector.tensor_scalar_mul(U_sb[:], U_sb[:], rsum[:])
                            nc.sync.dma_start(attn_sh[bb, h, ts(qt, 128), :], U_sb[:])
                            at_tiles[h].append(U_sb)
                    for kt in range(nk // 128):
                        aTs = []
                        for h in range(hpc):
                            ps_t = pstr.tile([128, QG * 128], BF, tag="pst", name=f"pst{h}")
                            for qi in range(QG):
                                nc.tensor.transpose(
                                    ps_t[:, ts(qi, 128)],
                                    at_tiles[h][qi][:, ts(kt, 128)], id_sb[:])
                            aT_sb = aTpool.tile([128, QG * 128], BF, tag="aTt", name=f"aT{h}")
                            if h % 2 == 0:
                                nc.vector.tensor_copy(aT_sb[:], ps_t[:])
                            else:
                                nc.scalar.copy(aT_sb[:], ps_t[:])
                            aTs.append(aT_sb)
                        for h in range(hpc):
                            nc.tensor.matmul(
                                ps_o2[bass.ds(h * dh, dh), :],
                                V_sb[:, bass.ds((bb * nk // 128 + kt) * 128 + h * dh, dh)],
                                aTs[h][:],
                                start=(kt == 0), stop=(kt == nk // 128 - 1),
                                tile_position=(0, h * dh), skip_group_check=True)
                    OTg = OT_sb[:, bass.ds(bb * nq + qg * QG * 128, QG * 128)]
                    nc.vector.tensor_copy(OTg[:], ps_o2[:])
                    for qi in range(QG):
                        qt = qg * QG + qi
                        for oc in range(d // OC):
                            ps_p = pso.tile([128, OC], FP32, tag="pso", name="ps_p")
                            nc.tensor.matmul(ps_p[:], OTg[:, ts(qi, 128)],
                                             wo_sb[:, ts(oc, OC)], start=True, stop=True)
                            pp = ppool.tile([128, OC], BF, tag="pp")
                            nc.scalar.copy(pp[:], ps_p[:])
                            nc.sync.dma_start(partial[bb, ts(qt, 128), ts(oc, OC)], pp[:])

    nc.compile()
    return nc


def _shard_inputs(query, key, value, vessel_bias, Wq, bq, Wk, bk, Wv, bv, Wo, bo,
                  b=B, nq=NQ, nk=NK, d=D, hpc=HPC, dh=DH):
    """Host-side prep: transpose/cast/slice per-core operands."""
    scale = 1.0 / math.sqrt(dh)
    qT = np.ascontiguousarray(query.reshape(b * nq, d).T).astype(BF16)
    kT = np.ascontiguousarray(key.reshape(b * nk, d).T).astype(BF16)
    vT = np.ascontiguousarray(value.reshape(b * nk, d).T).astype(BF16)
    biasn = vessel_bias.astype(BF16)
    ident = np.eye(128, dtype=BF16)
    in_maps = []
    for c in range(N_CORES):
        rows = slice(c * hpc * dh, (c + 1) * hpc * dh)
        in_maps.append({
            "qT": qT, "kT": kT, "vT": vT, "biasn": biasn, "ident": ident,
            "wqT": np.ascontiguousarray((Wq[rows] * scale).T).astype(BF16),
            "wkT": np.ascontiguousarray(Wk[rows].T).astype(BF16),
            "wvT": np.ascontiguousarray(Wv[rows].T).astype(BF16),
            "bq": (bq[rows] * scale).astype(np.float32).reshape(128, 1),
            "bk": bk[rows].astype(np.float32).reshape(128, 1),
            "bv": bv[rows].astype(BF16).reshape(1, 128),
            "woT": np.ascontiguousarray(Wo[:, rows].T).astype(BF16),
        })
    return in_maps


_NC_CACHE = {}


def _get_nc():
    if "nc" not in _NC_CACHE:
        _NC_CACHE["nc"] = build_nc()
    return _NC_CACHE["nc"]


def kernel(query, key, value, vessel_bias, Wq, bq, Wk, bk, Wv, bv, Wo, bo,
           _trace=False):
    query = np.asarray(query, dtype=np.float32)
    key = np.asarray(key, dtype=np.float32)
    value = np.asarray(value, dtype=np.float32)
    vessel_bias = np.asarray(vessel_bias, dtype=np.float32)
    nc = _get_nc()
    in_maps = _shard_inputs(query, key, value, vessel_bias,
                            np.asarray(Wq, np.float32), np.asarray(bq, np.float32),
                            np.asarray(Wk, np.float32), np.asarray(bk, np.float32),
                            np.asarray(Wv, np.float32), np.asarray(bv, np.float32),
                            np.asarray(Wo, np.float32), np.asarray(bo, np.float32))
    res = run_bass_kernel_spmd(nc, in_maps, list(range(N_CORES)), trace=_trace)

    attn = np.empty((B, H, NQ, NK), np.float32)
    out = np.zeros((B, NQ, D), np.float32)
    out += np.asarray(bo, np.float32)
    for c in range(N_CORES):
        attn[:, c * HPC:(c + 1) * HPC] = res.results[c]["attn_sh"].astype(np.float32)
        out += res.results[c]["partial"].astype(np.float32)
    if _trace:
        return (out, attn), res
    return (out, attn)


# revision 16
# speedup vs baseline: 1.3227x; 1.3227x over previous
"""Trainium2 Bass kernel for AnatomicalBiasedAttention.

Reference computation (fp32, B=2, NQ=NK=2048, D=1024, H=16, DH=64):
    Q = query @ Wq.T + bq ; K = key @ Wk.T + bk ; V = value @ Wv.T + bv
    scores = QK^T/sqrt(DH) + vessel_bias  (bias broadcast over heads)
    attn = softmax(scores, -1)
    out = (attn @ V) @ Wo.T + bo
    returns (out, attn)

Sharding: tensor-parallel over heads, 2 heads per core on 8 NeuronCores.
Each core projects its 2 heads' Q/K/V (pre-transposed bf16 operands are
prepared on the host), computes biased softmax attention for its heads,
writes its attn shard, and produces a partial output projection; the host
sums the 8 partials (TP unshard) and concatenates attn shards.
"""

import math
import numpy as np
import ml_dtypes
from contextlib import ExitStack

import concourse.bass as bass
import concourse.tile as tile
from concourse import bacc, mybir
from concourse.bass_utils import run_bass_kernel_spmd

BF16 = ml_dtypes.bfloat16
FP32 = mybir.dt.float32
BF = mybir.dt.bfloat16
AF = mybir.ActivationFunctionType
ts = bass.ts

N_CORES = 8
B = 2
NQ = 2048
NK = 2048
D = 1024
H = 16
DH = 64
HPC = H // N_CORES  # heads per core = 2


def build_nc(b=B, nq=NQ, nk=NK, d=D, hpc=HPC, dh=DH):
    """Build the per-core Bass graph (SPMD: all 8 cores run this graph).

    DRAM parameters (per-core shards, prepared by the host):
      qT, kT, vT   [d, b*ntok]  bf16   full transposed activations (replicated)
      wqT, wkT, wvT [d, hpc*dh] bf16   per-core head-slice of W.T (wqT pre-scaled 1/sqrt(dh))
      bq, bk       [hpc*dh, 1]  f32    per-core bias slices (bq pre-scaled)
      bv           [1, hpc*dh]  bf16
      biasn        [b, nq, nk]  bf16   vessel bias, natural layout (replicated)
      woT          [hpc*dh, d]  bf16   per-core rows of Wo.T
      bo8          [1, d]       bf16   bo / n_cores
      ident        [128, 128]   bf16   identity matrix
    Outputs:
      attn_sh [b, hpc, nq, nk] bf16
      partial [b, nq, d]       bf16
    """
    assert hpc * dh == 128
    NTOK_Q = b * nq
    NTOK_K = b * nk
    DT = d // 128       # D tiles
    TCH = 512           # token chunk for projections
    KC = min(512, nk)   # k chunk for scores
    OC = min(512, d)    # out-proj column chunk

    nc = bacc.Bacc("TRN2", target_bir_lowering=False, debug=False, num_devices=N_CORES)

    qT = nc.dram_tensor("qT", [d, NTOK_Q], BF, kind="ExternalInput").ap()
    kT = nc.dram_tensor("kT", [d, NTOK_K], BF, kind="ExternalInput").ap()
    vT = nc.dram_tensor("vT", [d, NTOK_K], BF, kind="ExternalInput").ap()
    wqT = nc.dram_tensor("wqT", [d, 128], BF, kind="ExternalInput").ap()
    wkT = nc.dram_tensor("wkT", [d, 128], BF, kind="ExternalInput").ap()
    wvT = nc.dram_tensor("wvT", [d, 128], BF, kind="ExternalInput").ap()
    bq = nc.dram_tensor("bq", [128, 1], FP32, kind="ExternalInput").ap()
    bk = nc.dram_tensor("bk", [128, 1], FP32, kind="ExternalInput").ap()
    bv = nc.dram_tensor("bv", [1, 128], BF, kind="ExternalInput").ap()
    biasn = nc.dram_tensor("biasn", [b, nq, nk], BF, kind="ExternalInput").ap()
    woT = nc.dram_tensor("woT", [128, d], BF, kind="ExternalInput").ap()
    ident = nc.dram_tensor("ident", [128, 128], BF, kind="ExternalInput").ap()

    attn_sh = nc.dram_tensor("attn_sh", [b, hpc, nq, nk], BF, kind="ExternalOutput").ap()
    partial = nc.dram_tensor("partial", [b, nq, d], BF, kind="ExternalOutput").ap()

    with tile.TileContext(nc) as tc, ExitStack() as ctx:
        # ---- constants resident in SBUF ----
        cpool = ctx.enter_context(tc.tile_pool(name="consts", bufs=1))
        wq_sb = cpool.tile([128, DT * 128], BF, tag="wq")
        nc.sync.dma_start(wq_sb[:].rearrange("p (a f) -> p a f", f=128),
                          wqT.rearrange("(a p) f -> p a f", p=128))
        wk_sb = cpool.tile([128, DT * 128], BF, tag="wk")
        nc.sync.dma_start(wk_sb[:].rearrange("p (a f) -> p a f", f=128),
                          wkT.rearrange("(a p) f -> p a f", p=128))
        wv_sb = cpool.tile([128, DT * 128], BF, tag="wv")
        nc.sync.dma_start(wv_sb[:].rearrange("p (a f) -> p a f", f=128),
                          wvT.rearrange("(a p) f -> p a f", p=128))
        wo_sb = cpool.tile([128, d], BF, tag="wo")
        nc.sync.dma_start(wo_sb[:], woT[:])
        bq_sb = cpool.tile([128, 1], FP32, tag="bq")
        nc.sync.dma_start(bq_sb[:], bq[:])
        bk_sb = cpool.tile([128, 1], FP32, tag="bk")
        nc.sync.dma_start(bk_sb[:], bk[:])
        bv_sb = cpool.tile([1, 128], BF, tag="bv")
        nc.sync.dma_start(bv_sb[:], bv[:])
        id_sb = cpool.tile([128, 128], BF, tag="ident")
        nc.sync.dma_start(id_sb[:], ident[:])
        ones_sb = cpool.tile([1, 128], BF, tag="ones")
        nc.vector.memset(ones_sb[:], 1.0)

        # persistent activations (bf16)
        apool = ctx.enter_context(tc.tile_pool(name="acts", bufs=1))
        QT_sb = apool.tile([128, NTOK_Q], BF, tag="QT")    # [2 heads x dh, tokens]
        KT_sb = apool.tile([128, NTOK_K], BF, tag="KT")
        V_sb = apool.tile([128, NTOK_K], BF, tag="V")      # natural: block t = tokens 128t..
        OT_sb = apool.tile([128, NTOK_Q], BF, tag="OT")    # attn-out^T, rows=2*dh concat

        # ---- stage A: projections ----
        with ExitStack() as sa:
            inp = sa.enter_context(tc.tile_pool(name="proj_in", bufs=2))
            psa = sa.enter_context(tc.tile_pool(name="proj_ps", bufs=2, space="PSUM"))
            psv = sa.enter_context(tc.tile_pool(name="proj_psv", bufs=2, space="PSUM"))
            for tch in range(max(NTOK_Q, NTOK_K) // TCH):
                if tch < NTOK_Q // TCH:
                    qin = inp.tile([128, DT * TCH], BF, tag="qin")
                    nc.sync.dma_start(
                        qin[:].rearrange("p (a f) -> p a f", a=DT),
                        qT[:, ts(tch, TCH)].rearrange("(a p) f -> p a f", p=128))
                    ps_q = psa.tile([128, TCH], FP32, tag="psq")
                    for dt_ in range(DT):
                        nc.tensor.matmul(ps_q[:], wq_sb[:, ts(dt_, 128)],
                                         qin[:, ts(dt_, TCH)],
                                         start=(dt_ == 0), stop=(dt_ == DT - 1))
                    nc.vector.tensor_scalar_add(QT_sb[:, ts(tch, TCH)], ps_q[:], bq_sb[:])
                if tch >= NTOK_K // TCH:
                    continue
                kin = inp.tile([128, DT * TCH], BF, tag="kin")
                nc.sync.dma_start(
                    kin[:].rearrange("p (a f) -> p a f", a=DT),
                    kT[:, ts(tch, TCH)].rearrange("(a p) f -> p a f", p=128))
                vin = inp.tile([128, DT * TCH], BF, tag="vin")
                nc.sync.dma_start(
                    vin[:].rearrange("p (a f) -> p a f", a=DT),
                    vT[:, ts(tch, TCH)].rearrange("(a p) f -> p a f", p=128))

                ps_k = psa.tile([128, TCH], FP32, tag="psq")
                for dt_ in range(DT):
                    nc.tensor.matmul(ps_k[:], wk_sb[:, ts(dt_, 128)],
                                     kin[:, ts(dt_, TCH)],
                                     start=(dt_ == 0), stop=(dt_ == DT - 1))
                nc.vector.tensor_scalar_add(KT_sb[:, ts(tch, TCH)], ps_k[:], bk_sb[:])

                ps_v = psv.tile([128, TCH], FP32, tag="psv")
                for tb in range(TCH // 128):
                    for dt_ in range(DT):
                        nc.tensor.matmul(ps_v[:, ts(tb, 128)],
                                         vin[:, bass.ds(dt_ * TCH + tb * 128, 128)],
                                         wv_sb[:, ts(dt_, 128)],
                                         start=(dt_ == 0), stop=False)
                    nc.tensor.matmul(ps_v[:, ts(tb, 128)], ones_sb[:], bv_sb[:],
                                     start=False, stop=True)
                nc.vector.tensor_copy(V_sb[:, ts(tch, TCH)], ps_v[:])

        # ---- stage B: attention ----
        # scores = QK^T/sqrt(dh) in PSUM; softmax via exp(s)*exp(bias)
        # (host supplies eb = exp(vessel_bias)); sums via fused accum_out.
        # attn@V consumes PE-transposed normalized attn in 512-wide groups.
        with ExitStack() as sb:
            bpool = sb.enter_context(tc.tile_pool(name="biasp", bufs=6))
            epool = sb.enter_context(tc.tile_pool(name="epool", bufs=4))
            atpool = sb.enter_context(tc.tile_pool(name="atpool", bufs=5))
            smpool = sb.enter_context(tc.tile_pool(name="smalls", bufs=8))
            aTpool = sb.enter_context(tc.tile_pool(name="aT", bufs=2))
            ppool = sb.enter_context(tc.tile_pool(name="pp", bufs=4))
            pss = sb.enter_context(tc.tile_pool(name="ps_s", bufs=2, space="PSUM"))
            pstr = sb.enter_context(tc.tile_pool(name="ps_tr", bufs=2, space="PSUM"))
            pso = sb.enter_context(tc.tile_pool(name="ps_o", bufs=2, space="PSUM"))
            QG = min(4, nq // 128)  # q-tiles per attn@V group
            for bb in range(b):
                for qg in range(nq // (128 * QG)):
                    eb_tiles = []
                    for qi in range(QG):
                        qt = qg * QG + qi
                        ebt = bpool.tile([128, nk], BF, tag="bias")
                        nc.sync.dma_start(ebt[:], biasn[bb, ts(qt, 128), :])
                        eb_tiles.append(ebt)
                    ps_o2 = pso.tile([128, QG * 128], FP32, tag="pso")
                    at_tiles = [[] for _ in range(hpc)]
                    SH = min(1024, nk)  # psum half-row width
                    for qi in range(QG):
                        qt = qg * QG + qi
                        # row-packed scores: both heads issued adjacently so the
                        # K=64 matmuls can run concurrently on row-groups 0-1 / 2-3
                        E_tiles = [epool.tile([128, nk], BF, tag="E", name=f"E{_h}") for _h in range(hpc)]
                        for sh in range(nk // SH):
                            ps_h = [pss.tile([128, SH], FP32, tag="pss", name=f"psh{_h}") for _h in range(hpc)]
                            for kc in range(SH // KC):
                                for h in range(hpc):
                                    nc.tensor.matmul(
                                        ps_h[h][:, ts(kc, KC)],
                                        QT_sb[bass.ds(h * dh, dh), bass.ds(bb * nq + qt * 128, 128)],
                                        KT_sb[bass.ds(h * dh, dh), bass.ds(bb * nk + sh * SH + kc * KC, KC)],
                                        start=True, stop=True)
                            for h in range(hpc):
                                nc.scalar.activation(E_tiles[h][:, ts(sh, SH)], ps_h[h][:], AF.Exp)
                        for h in range(hpc):
                            U_sb = atpool.tile([128, nk], BF, tag=f"at{h}")
                            ssum = smpool.tile([128, 1], FP32, tag="ssum")
                            nc.vector.scalar_tensor_tensor(
                                U_sb[:], E_tiles[h][:], 1.0, eb_tiles[qi][:],
                                op0=mybir.AluOpType.bypass, op1=mybir.AluOpType.mult,
                                accum_out=ssum[:])
                            rsum = smpool.tile([128, 1], FP32, tag="rsum")
                            nc.vector.reciprocal(rsum[:], ssum[:])
                            nc.vector.tensor_scalar_mul(U_sb[:], U_sb[:], rsum[:])
                            nc.sync.dma_start(attn_sh[bb, h, ts(qt, 128), :], U_sb[:])
                            at_tiles[h].append(U_sb)
                    for kt in range(nk // 128):
                        aTs = []
                        for h in range(hpc):
                            ps_t = pstr.tile([128, QG * 128], BF, tag="pst", name=f"pst{h}")
                            for qi in range(QG):
                                nc.tensor.transpose(
                                    ps_t[:, ts(qi, 128)],
                                    at_tiles[h][qi][:, ts(kt, 128)], id_sb[:])
                            aT_sb = aTpool.tile([128, QG * 128], BF, tag="aTt", name=f"aT{h}")
                            if h % 2 == 0:
                                nc.vector.tensor_copy(aT_sb[:], ps_t[:])
                            else:
                                nc.scalar.copy(aT_sb[:], ps_t[:])
                            aTs.append(aT_sb)
                        for h in range(hpc):
                            nc.tensor.matmul(
                                ps_o2[bass.ds(h * dh, dh), :],
                                V_sb[:, bass.ds((bb * nk // 128 + kt) * 128 + h * dh, dh)],
                                aTs[h][:],
                                start=(kt == 0), stop=(kt == nk // 128 - 1),
                                tile_position=(0, h * dh), skip_group_check=True)
                    OTg = OT_sb[:, bass.ds(bb * nq + qg * QG * 128, QG * 128)]
                    nc.vector.tensor_copy(OTg[:], ps_o2[:])
                    for qi in range(QG):
                        qt = qg * QG + qi
                        for oc in range(d // OC):
                            ps_p = pso.tile([128, OC], FP32, tag="pso", name="ps_p")
                            nc.tensor.matmul(ps_p[:], OTg[:, ts(qi, 128)],
                                             wo_sb[:, ts(oc, OC)], start=True, stop=True)
                            pp = ppool.tile([128, OC], BF, tag="pp")
                            nc.scalar.copy(pp[:], ps_p[:])
                            nc.sync.dma_start(partial[bb, ts(qt, 128), ts(oc, OC)], pp[:])

    nc.compile()
    return nc


def _shard_inputs(query, key, value, vessel_bias, Wq, bq, Wk, bk, Wv, bv, Wo, bo,
                  b=B, nq=NQ, nk=NK, d=D, hpc=HPC, dh=DH):
    """Host-side prep: transpose/cast/slice per-core operands."""
    scale = 1.0 / math.sqrt(dh)
    qT = np.ascontiguousarray(query.reshape(b * nq, d).T).astype(BF16)
    kT = np.ascontiguousarray(key.reshape(b * nk, d).T).astype(BF16)
    vT = np.ascontiguousarray(value.reshape(b * nk, d).T).astype(BF16)
    biasn = np.exp(vessel_bias).astype(BF16)
    ident = np.eye(128, dtype=BF16)
    in_maps = []
    for c in range(N_CORES):
        rows = slice(c * hpc * dh, (c + 1) * hpc * dh)
        in_maps.append({
            "qT": qT, "kT": kT, "vT": vT, "biasn": biasn, "ident": ident,
            "wqT": np.ascontiguousarray((Wq[rows] * scale).T).astype(BF16),
            "wkT": np.ascontiguousarray(Wk[rows].T).astype(BF16),
            "wvT": np.ascontiguousarray(Wv[rows].T).astype(BF16),
            "bq": (bq[rows] * scale).astype(np.float32).reshape(128, 1),
            "bk": bk[rows].astype(np.float32).reshape(128, 1),
            "bv": bv[rows].astype(BF16).reshape(1, 128),
            "woT": np.ascontiguousarray(Wo[:, rows].T).astype(BF16),
        })
    return in_maps


_NC_CACHE = {}


def _get_nc():
    if "nc" not in _NC_CACHE:
        _NC_CACHE["nc"] = build_nc()
    return _NC_CACHE["nc"]


def kernel(query, key, value, vessel_bias, Wq, bq, Wk, bk, Wv, bv, Wo, bo,
           _trace=False):
    query = np.asarray(query, dtype=np.float32)
    key = np.asarray(key, dtype=np.float32)
    value = np.asarray(value, dtype=np.float32)
    vessel_bias = np.asarray(vessel_bias, dtype=np.float32)
    nc = _get_nc()
    in_maps = _shard_inputs(query, key, value, vessel_bias,
                            np.asarray(Wq, np.float32), np.asarray(bq, np.float32),
                            np.asarray(Wk, np.float32), np.asarray(bk, np.float32),
                            np.asarray(Wv, np.float32), np.asarray(bv, np.float32),
                            np.asarray(Wo, np.float32), np.asarray(bo, np.float32))
    res = run_bass_kernel_spmd(nc, in_maps, list(range(N_CORES)), trace=_trace)

    attn = np.empty((B, H, NQ, NK), np.float32)
    out = np.zeros((B, NQ, D), np.float32)
    out += np.asarray(bo, np.float32)
    for c in range(N_CORES):
        attn[:, c * HPC:(c + 1) * HPC] = res.results[c]["attn_sh"].astype(np.float32)
        out += res.results[c]["partial"].astype(np.float32)
    if _trace:
        return (out, attn), res
    return (out, attn)


# revision 17
# speedup vs baseline: 1.3296x; 1.0052x over previous
"""Trainium2 Bass kernel for AnatomicalBiasedAttention.

Reference computation (fp32, B=2, NQ=NK=2048, D=1024, H=16, DH=64):
    Q = query @ Wq.T + bq ; K = key @ Wk.T + bk ; V = value @ Wv.T + bv
    scores = QK^T/sqrt(DH) + vessel_bias  (bias broadcast over heads)
    attn = softmax(scores, -1)
    out = (attn @ V) @ Wo.T + bo
    returns (out, attn)

Sharding: tensor-parallel over heads, 2 heads per core on 8 NeuronCores.
Each core projects its 2 heads' Q/K/V (pre-transposed bf16 operands are
prepared on the host), computes biased softmax attention for its heads,
writes its attn shard, and produces a partial output projection; the host
sums the 8 partials (TP unshard) and concatenates attn shards.
"""

import math
import numpy as np
import ml_dtypes
from contextlib import ExitStack

import concourse.bass as bass
import concourse.tile as tile
from concourse import bacc, mybir
from concourse.bass_utils import run_bass_kernel_spmd

BF16 = ml_dtypes.bfloat16
FP32 = mybir.dt.float32
BF = mybir.dt.bfloat16
AF = mybir.ActivationFunctionType
ts = bass.ts

N_CORES = 8
B = 2
NQ = 2048
NK = 2048
D = 1024
H = 16
DH = 64
HPC = H // N_CORES  # heads per core = 2


def build_nc(b=B, nq=NQ, nk=NK, d=D, hpc=HPC, dh=DH):
    """Build the per-core Bass graph (SPMD: all 8 cores run this graph).

    DRAM parameters (per-core shards, prepared by the host):
      qT, kT, vT   [d, b*ntok]  bf16   full transposed activations (replicated)
      wqT, wkT, wvT [d, hpc*dh] bf16   per-core head-slice of W.T (wqT pre-scaled 1/sqrt(dh))
      bq, bk       [hpc*dh, 1]  f32    per-core bias slices (bq pre-scaled)
      bv           [1, hpc*dh]  bf16
      biasn        [b, nq, nk]  bf16   vessel bias, natural layout (replicated)
      woT          [hpc*dh, d]  bf16   per-core rows of Wo.T
      bo8          [1, d]       bf16   bo / n_cores
      ident        [128, 128]   bf16   identity matrix
    Outputs:
      attn_sh [b, hpc, nq, nk] bf16
      partial [b, nq, d]       bf16
    """
    assert hpc * dh == 128
    NTOK_Q = b * nq
    NTOK_K = b * nk
    DT = d // 128       # D tiles
    TCH = 512           # token chunk for projections
    KC = min(512, nk)   # k chunk for scores
    OC = min(512, d)    # out-proj column chunk

    nc = bacc.Bacc("TRN2", target_bir_lowering=False, debug=False, num_devices=N_CORES)

    qT = nc.dram_tensor("qT", [d, NTOK_Q], BF, kind="ExternalInput").ap()
    kT = nc.dram_tensor("kT", [d, NTOK_K], BF, kind="ExternalInput").ap()
    vT = nc.dram_tensor("vT", [d, NTOK_K], BF, kind="ExternalInput").ap()
    wqT = nc.dram_tensor("wqT", [d, 128], BF, kind="ExternalInput").ap()
    wkT = nc.dram_tensor("wkT", [d, 128], BF, kind="ExternalInput").ap()
    wvT = nc.dram_tensor("wvT", [d, 128], BF, kind="ExternalInput").ap()
    bq = nc.dram_tensor("bq", [128, 1], FP32, kind="ExternalInput").ap()
    bk = nc.dram_tensor("bk", [128, 1], FP32, kind="ExternalInput").ap()
    bv = nc.dram_tensor("bv", [1, 128], BF, kind="ExternalInput").ap()
    biasn = nc.dram_tensor("biasn", [b, nq, nk], BF, kind="ExternalInput").ap()
    woT = nc.dram_tensor("woT", [128, d], BF, kind="ExternalInput").ap()
    ident = nc.dram_tensor("ident", [128, 128], BF, kind="ExternalInput").ap()

    attn_sh = nc.dram_tensor("attn_sh", [b, hpc, nq, nk], BF, kind="ExternalOutput").ap()
    partial = nc.dram_tensor("partial", [b, nq, d], BF, kind="ExternalOutput").ap()

    with tile.TileContext(nc) as tc, ExitStack() as ctx:
        # ---- constants resident in SBUF ----
        cpool = ctx.enter_context(tc.tile_pool(name="consts", bufs=1))
        wq_sb = cpool.tile([128, DT * 128], BF, tag="wq")
        nc.sync.dma_start(wq_sb[:].rearrange("p (a f) -> p a f", f=128),
                          wqT.rearrange("(a p) f -> p a f", p=128))
        wk_sb = cpool.tile([128, DT * 128], BF, tag="wk")
        nc.sync.dma_start(wk_sb[:].rearrange("p (a f) -> p a f", f=128),
                          wkT.rearrange("(a p) f -> p a f", p=128))
        wv_sb = cpool.tile([128, DT * 128], BF, tag="wv")
        nc.sync.dma_start(wv_sb[:].rearrange("p (a f) -> p a f", f=128),
                          wvT.rearrange("(a p) f -> p a f", p=128))
        wo_sb = cpool.tile([128, d], BF, tag="wo")
        nc.sync.dma_start(wo_sb[:], woT[:])
        bq_sb = cpool.tile([128, 1], FP32, tag="bq")
        nc.sync.dma_start(bq_sb[:], bq[:])
        bk_sb = cpool.tile([128, 1], FP32, tag="bk")
        nc.sync.dma_start(bk_sb[:], bk[:])
        bv_sb = cpool.tile([1, 128], BF, tag="bv")
        nc.sync.dma_start(bv_sb[:], bv[:])
        id_sb = cpool.tile([128, 128], BF, tag="ident")
        nc.sync.dma_start(id_sb[:], ident[:])
        ones_sb = cpool.tile([1, 128], BF, tag="ones")
        nc.vector.memset(ones_sb[:], 1.0)

        # persistent activations (bf16)
        apool = ctx.enter_context(tc.tile_pool(name="acts", bufs=1))
        QT_sb = apool.tile([128, NTOK_Q], BF, tag="QT")    # [2 heads x dh, tokens]
        KT_sb = apool.tile([128, NTOK_K], BF, tag="KT")
        V_sb = apool.tile([128, NTOK_K], BF, tag="V")      # natural: block t = tokens 128t..
        OT_sb = apool.tile([128, NTOK_Q], BF, tag="OT")    # attn-out^T, rows=2*dh concat

        # ---- stage A: projections ----
        with ExitStack() as sa:
            inp = sa.enter_context(tc.tile_pool(name="proj_in", bufs=2))
            psa = sa.enter_context(tc.tile_pool(name="proj_ps", bufs=2, space="PSUM"))
            psv = sa.enter_context(tc.tile_pool(name="proj_psv", bufs=2, space="PSUM"))
            for tch in range(max(NTOK_Q, NTOK_K) // TCH):
                if tch < NTOK_Q // TCH:
                    qin = inp.tile([128, DT * TCH], BF, tag="qin")
                    nc.sync.dma_start(
                        qin[:].rearrange("p (a f) -> p a f", a=DT),
                        qT[:, ts(tch, TCH)].rearrange("(a p) f -> p a f", p=128))
                    ps_q = psa.tile([128, TCH], FP32, tag="psq")
                    for dt_ in range(DT):
                        nc.tensor.matmul(ps_q[:], wq_sb[:, ts(dt_, 128)],
                                         qin[:, ts(dt_, TCH)],
                                         start=(dt_ == 0), stop=(dt_ == DT - 1))
                    nc.vector.tensor_scalar_add(QT_sb[:, ts(tch, TCH)], ps_q[:], bq_sb[:])
                if tch >= NTOK_K // TCH:
                    continue
                kin = inp.tile([128, DT * TCH], BF, tag="kin")
                nc.sync.dma_start(
                    kin[:].rearrange("p (a f) -> p a f", a=DT),
                    kT[:, ts(tch, TCH)].rearrange("(a p) f -> p a f", p=128))
                vin = inp.tile([128, DT * TCH], BF, tag="vin")
                nc.sync.dma_start(
                    vin[:].rearrange("p (a f) -> p a f", a=DT),
                    vT[:, ts(tch, TCH)].rearrange("(a p) f -> p a f", p=128))

                ps_k = psa.tile([128, TCH], FP32, tag="psq")
                for dt_ in range(DT):
                    nc.tensor.matmul(ps_k[:], wk_sb[:, ts(dt_, 128)],
                                     kin[:, ts(dt_, TCH)],
                                     start=(dt_ == 0), stop=(dt_ == DT - 1))
                nc.vector.tensor_scalar_add(KT_sb[:, ts(tch, TCH)], ps_k[:], bk_sb[:])

                ps_v = psv.tile([128, TCH], FP32, tag="psv")
                for tb in range(TCH // 128):
                    for dt_ in range(DT):
                        nc.tensor.matmul(ps_v[:, ts(tb, 128)],
                                         vin[:, bass.ds(dt_ * TCH + tb * 128, 128)],
                                         wv_sb[:, ts(dt_, 128)],
                                         start=(dt_ == 0), stop=False)
                    nc.tensor.matmul(ps_v[:, ts(tb, 128)], ones_sb[:], bv_sb[:],
                                     start=False, stop=True)
                nc.vector.tensor_copy(V_sb[:, ts(tch, TCH)], ps_v[:])

        # ---- stage B: attention ----
        # scores = QK^T/sqrt(dh) in PSUM; softmax via exp(s)*exp(bias)
        # (host supplies eb = exp(vessel_bias)); sums via fused accum_out.
        # attn@V consumes PE-transposed normalized attn in 512-wide groups.
        with ExitStack() as sb:
            bpool = sb.enter_context(tc.tile_pool(name="biasp", bufs=6))
            epool = sb.enter_context(tc.tile_pool(name="epool", bufs=4))
            atpool = sb.enter_context(tc.tile_pool(name="atpool", bufs=6))
            smpool = sb.enter_context(tc.tile_pool(name="smalls", bufs=16))
            aTpool = sb.enter_context(tc.tile_pool(name="aT", bufs=4))
            ppool = sb.enter_context(tc.tile_pool(name="pp", bufs=4))
            pss = sb.enter_context(tc.tile_pool(name="ps_s", bufs=2, space="PSUM"))
            pstr = sb.enter_context(tc.tile_pool(name="ps_tr", bufs=2, space="PSUM"))
            pso = sb.enter_context(tc.tile_pool(name="ps_o", bufs=2, space="PSUM"))
            QG = min(4, nq // 128)  # q-tiles per attn@V group
            for bb in range(b):
                for qg in range(nq // (128 * QG)):
                    eb_tiles = []
                    for qi in range(QG):
                        qt = qg * QG + qi
                        ebt = bpool.tile([128, nk], BF, tag="bias")
                        nc.sync.dma_start(ebt[:], biasn[bb, ts(qt, 128), :])
                        eb_tiles.append(ebt)
                    ps_o2 = pso.tile([128, QG * 128], FP32, tag="pso")
                    at_tiles = [[] for _ in range(hpc)]
                    SH = min(1024, nk)  # psum half-row width
                    for qi in range(QG):
                        qt = qg * QG + qi
                        # row-packed scores: both heads issued adjacently so the
                        # K=64 matmuls can run concurrently on row-groups 0-1 / 2-3
                        E_tiles = [epool.tile([128, nk], BF, tag="E", name=f"E{_h}") for _h in range(hpc)]
                        for sh in range(nk // SH):
                            ps_h = [pss.tile([128, SH], FP32, tag="pss", name=f"psh{_h}") for _h in range(hpc)]
                            for kc in range(SH // KC):
                                for h in range(hpc):
                                    nc.tensor.matmul(
                                        ps_h[h][:, ts(kc, KC)],
                                        QT_sb[bass.ds(h * dh, dh), bass.ds(bb * nq + qt * 128, 128)],
                                        KT_sb[bass.ds(h * dh, dh), bass.ds(bb * nk + sh * SH + kc * KC, KC)],
                                        start=True, stop=True)
                            for h in range(hpc):
                                nc.scalar.activation(E_tiles[h][:, ts(sh, SH)], ps_h[h][:], AF.Exp)
                        for h in range(hpc):
                            U_sb = atpool.tile([128, nk], BF, tag=f"at{h}")
                            ssum = smpool.tile([128, 1], FP32, tag="ssum")
                            nc.vector.scalar_tensor_tensor(
                                U_sb[:], E_tiles[h][:], 1.0, eb_tiles[qi][:],
                                op0=mybir.AluOpType.bypass, op1=mybir.AluOpType.mult,
                                accum_out=ssum[:])
                            rsum = smpool.tile([128, 1], FP32, tag="rsum")
                            nc.vector.reciprocal(rsum[:], ssum[:])
                            nc.vector.tensor_scalar_mul(U_sb[:], U_sb[:], rsum[:])
                            nc.sync.dma_start(attn_sh[bb, h, ts(qt, 128), :], U_sb[:])
                            at_tiles[h].append(U_sb)
                    for kt in range(nk // 128):
                        aTs = []
                        for h in range(hpc):
                            ps_t = pstr.tile([128, QG * 128], BF, tag="pst", name=f"pst{h}")
                            for qi in range(QG):
                                nc.tensor.transpose(
                                    ps_t[:, ts(qi, 128)],
                                    at_tiles[h][qi][:, ts(kt, 128)], id_sb[:])
                            aT_sb = aTpool.tile([128, QG * 128], BF, tag="aTt", name=f"aT{h}")
                            if h % 2 == 0:
                                nc.vector.tensor_copy(aT_sb[:], ps_t[:])
                            else:
                                nc.scalar.copy(aT_sb[:], ps_t[:])
                            aTs.append(aT_sb)
                        for h in range(hpc):
                            nc.tensor.matmul(
                                ps_o2[bass.ds(h * dh, dh), :],
                                V_sb[:, bass.ds((bb * nk // 128 + kt) * 128 + h * dh, dh)],
                                aTs[h][:],
                                start=(kt == 0), stop=(kt == nk // 128 - 1),
                                tile_position=(0, h * dh), skip_group_check=True)
                    OTg = OT_sb[:, bass.ds(bb * nq + qg * QG * 128, QG * 128)]
                    nc.vector.tensor_copy(OTg[:], ps_o2[:])
                    for qi in range(QG):
                        qt = qg * QG + qi
                        for oc in range(d // OC):
                            ps_p = pso.tile([128, OC], FP32, tag="pso", name="ps_p")
                            nc.tensor.matmul(ps_p[:], OTg[:, ts(qi, 128)],
                                             wo_sb[:, ts(oc, OC)], start=True, stop=True)
                            pp = ppool.tile([128, OC], BF, tag="pp")
                            nc.scalar.copy(pp[:], ps_p[:])
                            nc.sync.dma_start(partial[bb, ts(qt, 128), ts(oc, OC)], pp[:])

    nc.compile()
    return nc


def _shard_inputs(query, key, value, vessel_bias, Wq, bq, Wk, bk, Wv, bv, Wo, bo,
                  b=B, nq=NQ, nk=NK, d=D, hpc=HPC, dh=DH):
    """Host-side prep: transpose/cast/slice per-core operands."""
    scale = 1.0 / math.sqrt(dh)
    qT = np.ascontiguousarray(query.reshape(b * nq, d).T).astype(BF16)
    kT = np.ascontiguousarray(key.reshape(b * nk, d).T).astype(BF16)
    vT = np.ascontiguousarray(value.reshape(b * nk, d).T).astype(BF16)
    biasn = np.exp(vessel_bias).astype(BF16)
    ident = np.eye(128, dtype=BF16)
    in_maps = []
    for c in range(N_CORES):
        rows = slice(c * hpc * dh, (c + 1) * hpc * dh)
        in_maps.append({
            "qT": qT, "kT": kT, "vT": vT, "biasn": biasn, "ident": ident,
            "wqT": np.ascontiguousarray((Wq[rows] * scale).T).astype(BF16),
            "wkT": np.ascontiguousarray(Wk[rows].T).astype(BF16),
            "wvT": np.ascontiguousarray(Wv[rows].T).astype(BF16),
            "bq": (bq[rows] * scale).astype(np.float32).reshape(128, 1),
            "bk": bk[rows].astype(np.float32).reshape(128, 1),
            "bv": bv[rows].astype(BF16).reshape(1, 128),
            "woT": np.ascontiguousarray(Wo[:, rows].T).astype(BF16),
        })
    return in_maps


_NC_CACHE = {}


def _get_nc():
    if "nc" not in _NC_CACHE:
        _NC_CACHE["nc"] = build_nc()
    return _NC_CACHE["nc"]


def kernel(query, key, value, vessel_bias, Wq, bq, Wk, bk, Wv, bv, Wo, bo,
           _trace=False):
    query = np.asarray(query, dtype=np.float32)
    key = np.asarray(key, dtype=np.float32)
    value = np.asarray(value, dtype=np.float32)
    vessel_bias = np.asarray(vessel_bias, dtype=np.float32)
    nc = _get_nc()
    in_maps = _shard_inputs(query, key, value, vessel_bias,
                            np.asarray(Wq, np.float32), np.asarray(bq, np.float32),
                            np.asarray(Wk, np.float32), np.asarray(bk, np.float32),
                            np.asarray(Wv, np.float32), np.asarray(bv, np.float32),
                            np.asarray(Wo, np.float32), np.asarray(bo, np.float32))
    res = run_bass_kernel_spmd(nc, in_maps, list(range(N_CORES)), trace=_trace)

    attn = np.empty((B, H, NQ, NK), np.float32)
    out = np.zeros((B, NQ, D), np.float32)
    out += np.asarray(bo, np.float32)
    for c in range(N_CORES):
        attn[:, c * HPC:(c + 1) * HPC] = res.results[c]["attn_sh"].astype(np.float32)
        out += res.results[c]["partial"].astype(np.float32)
    if _trace:
        return (out, attn), res
    return (out, attn)
